# revision 9
# baseline (speedup 1.0000x reference)
"""Trainium2 Bass kernel for nn_Graph_to_Featuremaps_savemem.

Math: softmax over nodes is shift-invariant, so the (res @ nfr)[b,p] term
cancels and res_feature never affects the output:
    attn[b,p,:] = softmax(x[b] @ nfh)          (independent of p)
    out[b,c,h,w] = relu(((e_b^T x[b]) @ W)[c] / sum(e_b))   broadcast over (h,w)
with e_b = exp(x[b] @ nfh). The kernel is a tiny per-batch softmax-weighted
reduction followed by a huge broadcast write — pure HBM-write-bound, sharded
batch-parallel over 8 cores (2 batches/core).

Performance structure (per core):
  - Output is written in float16 (host upcasts): 16 MB instead of 32 MB.
    fp16 quantization adds ~3e-4 rms rel err, far inside the 2e-2 gate.
  - Inputs arrive as two packed bf16 DRAM buffers (pa: X^T|nfh on sync queue,
    pb: X|W on scalar queue) so one large-descriptor DMA per queue replaces
    three 512B-descriptor f32 loads. X^T is transposed on host, removing the
    on-device PE transpose from the critical path.
  - All matmuls run on bf16 inputs: single-pass (vs the two-pass fp32
    LOW/HIGH split), half the LDWEIGHTS bytes. Accumulation stays fp32 in
    PSUM; total rel err ~1e-2 worst case, inside the 2e-2 gate.
  - The per-(batch, c-half) fill tile [128, 4096] f16 is built by ACT and DVE
    in parallel; ACT fuses broadcast+normalize+relu in one op:
    activation(Relu, in=V broadcast, scale=1/sum_b).
  - Each 128-row output block is written by ONE dma_start whose source AP
    re-reads the fill tile 4x (stride-0 middle dim): 4 DMAs of 4 MB, 8 KB
    descriptors, split 2+2 over the sync/scalar HWDGE rings.
"""

import numpy as np

N_CORES = 8
B, NODES, HID, C, H, W = 16, 64, 128, 256, 128, 128
HWP = H * W  # 16384
B_LOC = B // N_CORES  # 2 batches per core
FILL_F = 16384  # fill tile free width; DMA repeats it HWP//FILL_F times
FILL0_F = 1024  # narrower first fill: earlier first output DMA
ACT_W = 4096  # columns of fills 1..3 computed by the ACT engine (rest: DVE)
PA_COLS = 256  # XT(128) | nfh(1) | pad to 512B/partition descriptors

_NC_CACHE = {}


def build_nc():
    import concourse.bass as bass
    import concourse.bacc as bacc
    import concourse.mybir as mybir
    from concourse.tile import TileContext

    f32 = mybir.dt.float32
    bf16 = mybir.dt.bfloat16
    f16 = mybir.dt.float16
    Alu = mybir.AluOpType
    Act = mybir.ActivationFunctionType

    nc = bacc.Bacc(None, target_bir_lowering=False, debug=False)
    # pa: X^T (cols 0:128) | nfh (col 128) | pad   -- critical-path inputs
    pa_d = nc.declare_dram_parameter("pa", [128, PA_COLS], bf16, isOutput=False)
    # pb: X (cols 0:128) | W (cols 128:384)
    pb_d = nc.declare_dram_parameter("pb", [128, HID + C], bf16, isOutput=False)
    out_d = nc.declare_dram_parameter("out", [B_LOC * C, HWP], f16, isOutput=True)

    def bcast(ap, n):
        # (P,1) AP -> (P,n) AP re-reading the same element along free dim
        return type(ap)(ap.tensor, ap.offset, [list(ap.ap[0]), [0, n]])

    def rep(ap, n):
        # (P,F) AP -> (P,n,F) AP re-reading the whole tile n times
        return type(ap)(ap.tensor, ap.offset, [list(ap.ap[0]), [0, n], list(ap.ap[1])])

    with TileContext(nc) as tc:
        with (
            nc.allow_low_precision(reason="fp16 output within 2e-2 rel-err gate"),
            tc.tile_pool(name="singles", bufs=1) as singles,
            tc.tile_pool(name="fills", bufs=1) as fills,
            tc.tile_pool(name="psum", bufs=4, space="PSUM") as psum,
            tc.tile_pool(name="psumv", bufs=1, space="PSUM") as psumv,
        ):
            # ---- constants (no input deps; DVE, overlap the input DMAs) ----
            MASK2 = singles.tile([128, 2], bf16, tag="MASK2")
            nc.vector.memset(MASK2[:], 0.0)
            nc.vector.memset(MASK2[0:64, 0:1], 1.0)
            nc.vector.memset(MASK2[64:128, 1:2], 1.0)
            ONES1 = singles.tile([1, 128], bf16, tag="ONES1")
            nc.vector.memset(ONES1[:], 1.0)

            # ---- packed input loads (pa on sync ring, pb on scalar ring) ----
            PA = singles.tile([128, PA_COLS], bf16, tag="PA")
            nc.sync.dma_start(out=PA[:], in_=pa_d[:])
            PB = singles.tile([128, HID + C], bf16, tag="PB")
            nc.scalar.dma_start(out=PB[:], in_=pb_d[:])

            XT = PA[:, 0:HID]
            NFH = PA[:, HID : HID + 1]
            X = PB[:, 0:HID]
            Wt = PB[:, HID : HID + C]

            # ---- s = X @ nfh (as column), e = exp(s) ----
            s_ps = psum.tile([128, 1], f32, tag="ps")
            nc.tensor.matmul(s_ps[:], XT, NFH)
            e_col = singles.tile([128, 1], bf16, tag="e_col")
            nc.scalar.activation(e_col[:], s_ps[:], Act.Exp)

            # ---- per-batch sums (row [1,2] via mask matmul), reciprocals,
            #      broadcast to all partitions: RC[:, b] = 1/sum_b ----
            S2_ps = psum.tile([1, 2], f32, tag="ps")
            nc.tensor.matmul(S2_ps[:], e_col[:], MASK2[:])

            # U'[b] = X[b]^T @ e[b]  (PE busy-work while DVE does reciprocal)
            U_ps = [
                psum.tile([HID, 1], f32, tag="ps", name=f"U_ps{b}")
                for b in range(B_LOC)
            ]
            U_sb = [
                singles.tile([HID, 1], bf16, tag=f"U_sb{b}", name=f"U_sb{b}")
                for b in range(B_LOC)
            ]
            sl0 = slice(0, NODES)
            nc.tensor.matmul(U_ps[0][:], X[sl0, :], e_col[sl0, :])

            r_row = singles.tile([1, 2], bf16, tag="r_row")
            nc.vector.reciprocal(r_row[:], S2_ps[:])
            RC_ps = psum.tile([128, 2], f32, tag="ps")
            nc.tensor.matmul(RC_ps[:], ONES1[:], r_row[:])
            RC = singles.tile([128, 2], f32, tag="RC")
            nc.vector.tensor_copy(RC[:], RC_ps[:])

            nc.scalar.activation(U_sb[0][:], U_ps[0][:], Act.Copy)
            sl1 = slice(NODES, 2 * NODES)
            nc.tensor.matmul(U_ps[1][:], X[sl1, :], e_col[sl1, :])
            nc.scalar.activation(U_sb[1][:], U_ps[1][:], Act.Copy)

            # ---- per (batch, c-half): V' = W_h^T U', VR = relu(V'/sum) as a
            #      [128,1] column, fill tiles are broadcast copies of VR, and
            #      each 128-row output block is ONE whole-row DMA (repeat AP).
            #      All output DMAs ride the otherwise-idle sync engine. ----
            k = 0
            for b in range(B_LOC):
                for hf in range(C // 128):
                    V_ps = psumv.tile(
                        [128, 1], f32, tag=f"V_ps{b}{hf}", name=f"V_ps{b}{hf}"
                    )
                    nc.tensor.matmul(
                        V_ps[:], Wt[:, hf * 128 : (hf + 1) * 128], U_sb[b][:]
                    )
                    fw = FILL0_F if k == 0 else FILL_F
                    fill = fills.tile(
                        [128, fw], f16, tag=f"fill{b}{hf}", name=f"fill{b}{hf}"
                    )
                    # VR* = max(V * (1/sum_b), 0); separate source tiles per
                    # consumer engine so no cross-engine ordering can appear.
                    VRd = singles.tile(
                        [128, 1], f32, tag=f"VRd{b}{hf}", name=f"VRd{b}{hf}"
                    )
                    nc.vector.tensor_scalar(
                        VRd[:], V_ps[:], RC[:, b : b + 1], 0.0,
                        op0=Alu.mult, op1=Alu.max,
                    )
                    if k == 0:
                        # first fill: DVE-only, narrow, lowest latency
                        nc.vector.tensor_copy(fill[:, :], bcast(VRd[:], fw))
                    else:
                        VRa = singles.tile(
                            [128, 1], f32, tag=f"VRa{b}{hf}", name=f"VRa{b}{hf}"
                        )
                        nc.vector.tensor_scalar(
                            VRa[:], V_ps[:], RC[:, b : b + 1], 0.0,
                            op0=Alu.mult, op1=Alu.max,
                        )
                        nc.scalar.activation(
                            fill[:, 0:ACT_W], bcast(VRa[:], ACT_W), Act.Copy
                        )
                        nc.vector.tensor_copy(
                            fill[:, ACT_W:fw], bcast(VRd[:], fw - ACT_W)
                        )
                    r0 = (b * C + hf * 128)
                    nc.sync.dma_start(
                        out=out_d[r0 : r0 + 128, :], in_=rep(fill[:], HWP // fw)
                    )
                    k += 1
    nc.finalize()
    return nc


def get_nc():
    if "nc" not in _NC_CACHE:
        _NC_CACHE["nc"] = build_nc()
    return _NC_CACHE["nc"]


def make_in_maps(input, node_fea_for_hidden, weight):
    import ml_dtypes

    bf = ml_dtypes.bfloat16
    x = np.asarray(input, np.float32)[0]  # (B, NODES, HID)
    nfh = np.asarray(node_fea_for_hidden, np.float32).reshape(HID)
    w = np.asarray(weight, np.float32)  # (HID, C)
    in_maps = []
    for i in range(N_CORES):
        xs = x[i * B_LOC : (i + 1) * B_LOC].reshape(B_LOC * NODES, HID)
        pa = np.zeros((128, PA_COLS), bf)
        pa[:, 0:HID] = xs.T.astype(bf)
        pa[:, HID] = nfh.astype(bf)
        pb = np.empty((128, HID + C), bf)
        pb[:, 0:HID] = xs.astype(bf)
        pb[:, HID:] = w.astype(bf)
        in_maps.append(
            {"pa": np.ascontiguousarray(pa), "pb": np.ascontiguousarray(pb)}
        )
    return in_maps


def run_spmd(in_maps, trace=False, **kw):
    from concourse.bass_utils import run_bass_kernel_spmd

    return run_bass_kernel_spmd(get_nc(), in_maps, list(range(N_CORES)), trace=trace, **kw)


def kernel(input, res_feature, node_fea_for_res, node_fea_for_hidden, weight):
    res = run_spmd(make_in_maps(input, node_fea_for_hidden, weight)).results
    out = np.concatenate(
        [r["out"].reshape(B_LOC, C, H, W) for r in res], axis=0
    )
    return out.astype(np.float32)


# revision 11
# speedup vs baseline: 1.1582x; 1.1582x over previous
"""Trainium2 Bass kernel for nn_Graph_to_Featuremaps_savemem.

Math: softmax over nodes is shift-invariant, so the (res @ nfr)[b,p] term
cancels and res_feature never affects the output:
    attn[b,p,:] = softmax(x[b] @ nfh)          (independent of p)
    out[b,c,h,w] = relu(((e_b^T x[b]) @ W)[c] / sum(e_b))   broadcast over (h,w)
with e_b = exp(x[b] @ nfh). The kernel is a tiny per-batch softmax-weighted
reduction followed by a huge broadcast write — pure HBM-write-bound, sharded
batch-parallel over 8 cores (2 batches/core).

Performance structure (per core):
  - Output is written in float16 (host upcasts): 16 MB instead of 32 MB.
    fp16 quantization adds ~3e-4 rms rel err, far inside the 2e-2 gate.
  - Inputs arrive as two packed bf16 DRAM buffers (pa: X^T|nfh on sync queue,
    pb: X|W on scalar queue) so one large-descriptor DMA per queue replaces
    three 512B-descriptor f32 loads. X^T is transposed on host, removing the
    on-device PE transpose from the critical path.
  - All matmuls run on bf16 inputs: single-pass (vs the two-pass fp32
    LOW/HIGH split), half the LDWEIGHTS bytes. Accumulation stays fp32 in
    PSUM; total rel err ~1e-2 worst case, inside the 2e-2 gate.
  - The per-(batch, c-half) fill tile [128, 4096] f16 is built by ACT and DVE
    in parallel; ACT fuses broadcast+normalize+relu in one op:
    activation(Relu, in=V broadcast, scale=1/sum_b).
  - Each 128-row output block is written by ONE dma_start whose source AP
    re-reads the fill tile 4x (stride-0 middle dim): 4 DMAs of 4 MB, 8 KB
    descriptors, split 2+2 over the sync/scalar HWDGE rings.
"""

import numpy as np

N_CORES = 8
B, NODES, HID, C, H, W = 16, 64, 128, 256, 128, 128
HWP = H * W  # 16384
B_LOC = B // N_CORES  # 2 batches per core
FILL_F = 4096  # fill tile free width
FILL0_F = 4096  # first fill width
ACT_W = 1024  # columns of each fill computed by the ACT engine (rest: DVE)
N_SUB = 4  # separate DMAs per 128-row block (no repeat-AP)
PA_COLS = 256  # XT(128) | nfh(1) | pad to 512B/partition descriptors

_NC_CACHE = {}


def build_nc():
    import concourse.bass as bass
    import concourse.bacc as bacc
    import concourse.mybir as mybir
    from concourse.tile import TileContext

    f32 = mybir.dt.float32
    bf16 = mybir.dt.bfloat16
    f16 = mybir.dt.float16
    Alu = mybir.AluOpType
    Act = mybir.ActivationFunctionType

    nc = bacc.Bacc(None, target_bir_lowering=False, debug=False)
    # pa: X^T (cols 0:128) | nfh (col 128) | pad   -- critical-path inputs
    pa_d = nc.declare_dram_parameter("pa", [128, PA_COLS], bf16, isOutput=False)
    # pb: X (cols 0:128) | W (cols 128:384)
    pb_d = nc.declare_dram_parameter("pb", [128, HID + C], bf16, isOutput=False)
    out_d = nc.declare_dram_parameter("out", [B_LOC * C, HWP], f16, isOutput=True)

    def bcast(ap, n):
        # (P,1) AP -> (P,n) AP re-reading the same element along free dim
        return type(ap)(ap.tensor, ap.offset, [list(ap.ap[0]), [0, n]])

    def rep(ap, n):
        # (P,F) AP -> (P,n,F) AP re-reading the whole tile n times
        return type(ap)(ap.tensor, ap.offset, [list(ap.ap[0]), [0, n], list(ap.ap[1])])

    with TileContext(nc) as tc:
        with (
            nc.allow_low_precision(reason="fp16 output within 2e-2 rel-err gate"),
            tc.tile_pool(name="singles", bufs=1) as singles,
            tc.tile_pool(name="fills", bufs=1) as fills,
            tc.tile_pool(name="psum", bufs=4, space="PSUM") as psum,
            tc.tile_pool(name="psumv", bufs=1, space="PSUM") as psumv,
        ):
            # ---- constants (no input deps; DVE, overlap the input DMAs) ----
            MASK2 = singles.tile([128, 2], bf16, tag="MASK2")
            nc.vector.memset(MASK2[:], 0.0)
            nc.vector.memset(MASK2[0:64, 0:1], 1.0)
            nc.vector.memset(MASK2[64:128, 1:2], 1.0)
            ONES1 = singles.tile([1, 128], bf16, tag="ONES1")
            nc.vector.memset(ONES1[:], 1.0)

            # ---- packed input loads (pa on sync ring, pb on scalar ring) ----
            PA = singles.tile([128, PA_COLS], bf16, tag="PA")
            nc.sync.dma_start(out=PA[:], in_=pa_d[:])
            PB = singles.tile([128, HID + C], bf16, tag="PB")
            nc.scalar.dma_start(out=PB[:], in_=pb_d[:])

            XT = PA[:, 0:HID]
            NFH = PA[:, HID : HID + 1]
            X = PB[:, 0:HID]
            Wt = PB[:, HID : HID + C]

            # ---- s = X @ nfh (as column), e = exp(s) ----
            s_ps = psum.tile([128, 1], f32, tag="ps")
            nc.tensor.matmul(s_ps[:], XT, NFH)
            e_col = singles.tile([128, 1], bf16, tag="e_col")
            nc.scalar.activation(e_col[:], s_ps[:], Act.Exp)

            # ---- per-batch sums (row [1,2] via mask matmul), reciprocals,
            #      broadcast to all partitions: RC[:, b] = 1/sum_b ----
            S2_ps = psum.tile([1, 2], f32, tag="ps")
            nc.tensor.matmul(S2_ps[:], e_col[:], MASK2[:])

            # U'[b] = X[b]^T @ e[b]  (PE busy-work while DVE does reciprocal)
            U_ps = [
                psum.tile([HID, 1], f32, tag="ps", name=f"U_ps{b}")
                for b in range(B_LOC)
            ]
            U_sb = [
                singles.tile([HID, 1], bf16, tag=f"U_sb{b}", name=f"U_sb{b}")
                for b in range(B_LOC)
            ]
            sl0 = slice(0, NODES)
            nc.tensor.matmul(U_ps[0][:], X[sl0, :], e_col[sl0, :])

            r_row = singles.tile([1, 2], bf16, tag="r_row")
            nc.vector.reciprocal(r_row[:], S2_ps[:])
            RC_ps = psum.tile([128, 2], f32, tag="ps")
            nc.tensor.matmul(RC_ps[:], ONES1[:], r_row[:])
            RC = singles.tile([128, 2], f32, tag="RC")
            nc.vector.tensor_copy(RC[:], RC_ps[:])

            nc.scalar.activation(U_sb[0][:], U_ps[0][:], Act.Copy)
            sl1 = slice(NODES, 2 * NODES)
            nc.tensor.matmul(U_ps[1][:], X[sl1, :], e_col[sl1, :])
            nc.scalar.activation(U_sb[1][:], U_ps[1][:], Act.Copy)

            # ---- per (batch, c-half): V' = W_h^T U', VR = relu(V'/sum) as a
            #      [128,1] column, fill tiles are broadcast copies of VR, and
            #      each 128-row output block is ONE whole-row DMA (repeat AP).
            #      All output DMAs ride the otherwise-idle sync engine. ----
            k = 0
            for b in range(B_LOC):
                for hf in range(C // 128):
                    V_ps = psumv.tile(
                        [128, 1], f32, tag=f"V_ps{b}{hf}", name=f"V_ps{b}{hf}"
                    )
                    nc.tensor.matmul(
                        V_ps[:], Wt[:, hf * 128 : (hf + 1) * 128], U_sb[b][:]
                    )
                    fw = FILL0_F if k == 0 else FILL_F
                    fill = fills.tile(
                        [128, fw], f16, tag=f"fill{b}{hf}", name=f"fill{b}{hf}"
                    )
                    # VR* = max(V * (1/sum_b), 0); separate source tiles per
                    # consumer engine so no cross-engine ordering can appear.
                    VRd = singles.tile(
                        [128, 1], f32, tag=f"VRd{b}{hf}", name=f"VRd{b}{hf}"
                    )
                    nc.vector.tensor_scalar(
                        VRd[:], V_ps[:], RC[:, b : b + 1], 0.0,
                        op0=Alu.mult, op1=Alu.max,
                    )
                    VRa = singles.tile(
                        [128, 1], f32, tag=f"VRa{b}{hf}", name=f"VRa{b}{hf}"
                    )
                    nc.vector.tensor_scalar(
                        VRa[:], V_ps[:], RC[:, b : b + 1], 0.0,
                        op0=Alu.mult, op1=Alu.max,
                    )
                    nc.scalar.activation(
                        fill[:, 0:ACT_W], bcast(VRa[:], ACT_W), Act.Copy
                    )
                    nc.vector.tensor_copy(
                        fill[:, ACT_W:fw], bcast(VRd[:], fw - ACT_W)
                    )
                    r0 = (b * C + hf * 128)
                    for s in range(N_SUB):
                        nc.sync.dma_start(
                            out=out_d[r0 : r0 + 128, s * fw : (s + 1) * fw],
                            in_=fill[:],
                        )
                    k += 1
    nc.finalize()
    return nc


def get_nc():
    if "nc" not in _NC_CACHE:
        _NC_CACHE["nc"] = build_nc()
    return _NC_CACHE["nc"]


def make_in_maps(input, node_fea_for_hidden, weight):
    import ml_dtypes

    bf = ml_dtypes.bfloat16
    x = np.asarray(input, np.float32)[0]  # (B, NODES, HID)
    nfh = np.asarray(node_fea_for_hidden, np.float32).reshape(HID)
    w = np.asarray(weight, np.float32)  # (HID, C)
    in_maps = []
    for i in range(N_CORES):
        xs = x[i * B_LOC : (i + 1) * B_LOC].reshape(B_LOC * NODES, HID)
        pa = np.zeros((128, PA_COLS), bf)
        pa[:, 0:HID] = xs.T.astype(bf)
        pa[:, HID] = nfh.astype(bf)
        pb = np.empty((128, HID + C), bf)
        pb[:, 0:HID] = xs.astype(bf)
        pb[:, HID:] = w.astype(bf)
        in_maps.append(
            {"pa": np.ascontiguousarray(pa), "pb": np.ascontiguousarray(pb)}
        )
    return in_maps


def run_spmd(in_maps, trace=False, **kw):
    from concourse.bass_utils import run_bass_kernel_spmd

    return run_bass_kernel_spmd(get_nc(), in_maps, list(range(N_CORES)), trace=trace, **kw)


def kernel(input, res_feature, node_fea_for_res, node_fea_for_hidden, weight):
    res = run_spmd(make_in_maps(input, node_fea_for_hidden, weight)).results
    out = np.concatenate(
        [r["out"].reshape(B_LOC, C, H, W) for r in res], axis=0
    )
    return out.astype(np.float32)
